# revision 1
# baseline (speedup 1.0000x reference)
"""Trainium2 Bass kernel for a 2-layer relational GNN (ConceptGNN).

Math per layer (reference):
    msg  = x[src] * rel_emb[edge_attr]               # [E, D]
    agg  = segment_sum(msg, dst) / max(deg, 1)       # [N, D] mean
    h    = relu((x + agg) @ W)

Distribution: destination-sharded across 8 NeuronCores, 49 blocks of
128 dst slots per core. Nodes are dealt to (core, block, pos) slots by
descending degree so every block receives a near-equal edge load
(minimizes 128-edge chunk padding); the host inverts the permutation
when assembling outputs.

The host pre-gathers messages: slot (g, p) of chunk g carries the bf16
row x[src]∘rel_emb[attr]. Each block also gets one self-chunk whose
slots carry the block's own x rows with weight 1, so the residual x
needs no separate path and the dense phase is a single W matmul:
    h.T = relu(W.T @ aggT),   aggT = sum_chunks msgT_c @ S_c
Aggregation runs on the TensorEngine as a one-hot matmul: per chunk a
[128, 128] bf16 one-hot S[e, n] = (n == dst_e) is built with
tensor_scalar is_equal (w_e = 1/max(deg,1) is baked into the msg rows
on the host) and PE accumulates aggT += msg_c.T @ S_c into PSUM.
S-builds are split between the Vector and GpSimd engines to run both
in parallel. The dense matmul and relu run in f32; h.T is stored
transposed in bf16 and the host transposes back when assembling the
next layer's table / final output.

The same compiled program runs twice (layer 1 from x, layer 2 from h1);
edges are identical across layers so the chunk structure is shared.
"""

import numpy as np

# ---- problem constants (fixed by the harness contract) ----
N_NODES = 50000
N_EDGES = 640000
D = 128
N_REL = 6
NCORES = 8
P = 128

BLK = 128                 # dst-node block width (matmul moving dim)
NBLK = 49                 # blocks per core; 8*49*128 = 50176 >= 50000
SHARD = NBLK * BLK        # 6272 dst slots per core
STORE_BLKS = 7            # store h.T every 7 blocks (49 = 7*7)
POOL_FRAC = 0.33          # fraction of S-builds on the GpSimd engine
# msg-DMA queue assignment: Activation's copies biased early so its
# queue is clear near the tail; the rest split SP / GpSimd
ACT_BLOCKS = (2, 7, 12, 17, 22, 27, 32, 37)
POOL_BLOCKS = (3, 6, 10, 14, 18, 21, 25, 28, 31, 35, 38, 42)
LATE_DVE = True           # last two blocks' copy/relu on DVE


def _build_nc(chunks, *, d=D, nblk=NBLK, blk=BLK, ncores=NCORES):
    """Build the single-layer Bass program.

    chunks: per-block chunk counts (incl. the self chunk) — identical
    across cores (padded to the max) so one SPMD program serves all 8.
    """
    import concourse.bass as bass
    import concourse.bacc as bacc
    import concourse.mybir as mybir
    import concourse.tile as tile

    f32 = mybir.dt.float32
    bf16 = mybir.dt.bfloat16
    Act = mybir.ActivationFunctionType
    Alu = mybir.AluOpType

    chunks = [int(c) for c in chunks]
    G = sum(chunks)
    maxcb = max(chunks)

    nc = bacc.Bacc("TRN2", target_bir_lowering=False, debug=False,
                   num_devices=ncores)
    dmsg = nc.dram_tensor("dmsg", [P, G, d], bf16, kind="ExternalInput")
    drel = nc.dram_tensor("drel", [P, G], f32, kind="ExternalInput")
    dwm = nc.dram_tensor("dwm", [d, d], bf16, kind="ExternalInput")
    houtT = nc.dram_tensor("houtT", [P, nblk * blk], bf16,
                           kind="ExternalOutput")

    # msg DMAs are spread across the three DMA-capable queues (SP,
    # Activation, GpSimd) which the sim runs concurrently; S-builds are
    # split DVE/GpSimd. Fractions balance all five engines.
    act_blocks = set(ACT_BLOCKS)
    pool_blocks = set(POOL_BLOCKS)
    eng_pat = ["act" if b in act_blocks else
               "pool" if b in pool_blocks else "sp"
               for b in range(nblk)]

    with tile.TileContext(nc) as tc:
        with (
            tc.tile_pool(name="const", bufs=1) as cpool,
            tc.tile_pool(name="msg", bufs=12) as mpool,
            tc.tile_pool(name="onehot", bufs=24) as spool,
            tc.tile_pool(name="onehotp", bufs=12) as ppool,
            tc.tile_pool(name="agg", bufs=6) as apool,
            tc.tile_pool(name="psA", bufs=6, space="PSUM") as psA,
            tc.tile_pool(name="psH", bufs=2, space="PSUM") as psH,
        ):
            iot = cpool.tile([P, blk], bf16, tag="iota")
            nc.gpsimd.iota(iot[:], [[1, blk]], channel_multiplier=0,
                           allow_small_or_imprecise_dtypes=True)
            pvec = cpool.tile([P, 1], f32, tag="pvec")
            nc.gpsimd.iota(pvec[:], [[0, 1]], channel_multiplier=1,
                           allow_small_or_imprecise_dtypes=True)
            # every block's self chunk uses the same identity one-hot
            ident = cpool.tile([P, blk], bf16, tag="ident")
            nc.vector.tensor_scalar(out=ident[:], in0=iot[:],
                                    scalar1=pvec[:, 0:1], scalar2=None,
                                    op0=Alu.is_equal)
            hTall = cpool.tile([P, nblk * blk], bf16, tag="hT")
            drt = cpool.tile([P, G], f32, tag="drel")
            wmt = cpool.tile([P, d], bf16, tag="wm")

            gcol = np.concatenate([[0], np.cumsum(chunks)]).tolist()
            engs = {"sp": nc.sync, "act": nc.scalar, "pool": nc.gpsimd}
            mtiles = {}

            def load_msg(b, engine=None):
                cb = chunks[b]
                col = gcol[b]
                mg = mpool.tile([P, maxcb, d], bf16, tag="mg")
                e = engs[engine or eng_pat[b]]
                e.dma_start(mg[:, :cb, :], dmsg[:, col:col + cb, :])
                mtiles[b] = mg

            # head: a minimum-cost slice of drel covering ~23 blocks
            # lands first (Act), block-0 msg in two pieces on SP, the
            # rest of drel on GpSimd; wm early on Act
            dh = min(324, G)
            nc.scalar.dma_start(drt[:, :dh], drel[:, :dh])
            nc.scalar.dma_start(wmt[:], dwm[:])
            mg0a = cpool.tile([P, 4, d], bf16, tag="mg0a")
            nc.sync.dma_start(mg0a[:], dmsg[:, :4, :])
            c0 = int(chunks[0])
            mg0b = cpool.tile([P, maxcb - 4, d], bf16, tag="mg0b")
            nc.sync.dma_start(mg0b[:, :c0 - 4, :], dmsg[:, 4:c0, :])
            mtiles[0] = ("split", mg0a, mg0b)
            nc.gpsimd.dma_start(drt[:, dh:], drel[:, dh:])
            LA = 9
            for b in range(1, LA):
                load_msg(b)

            for b in range(nblk):
                if LA + b < nblk:
                    load_msg(LA + b)
                cb = chunks[b]
                col = gcol[b]
                n_pool = int(round(cb * POOL_FRAC))
                mg = mtiles.pop(b)
                split0 = isinstance(mg, tuple)
                ps = psA.tile([P, blk], f32, tag="psA")
                for c in range(cb):
                    if c == 0:
                        S = ident     # self chunk: identity one-hot
                    elif c >= cb - n_pool:
                        S = ppool.tile([P, blk], bf16, tag="Sp")
                        nc.gpsimd.tensor_scalar(
                            out=S[:], in0=iot[:],
                            scalar1=drt[:, col + c:col + c + 1],
                            scalar2=None, op0=Alu.is_equal)
                    else:
                        S = spool.tile([P, blk], bf16, tag="S")
                        nc.vector.tensor_scalar(
                            out=S[:], in0=iot[:],
                            scalar1=drt[:, col + c:col + c + 1],
                            scalar2=None, op0=Alu.is_equal)
                    if split0:
                        lhsT = (mg[1][:, c, :] if c < 4
                                else mg[2][:, c - 4, :])
                    else:
                        lhsT = mg[:, c, :]
                    nc.tensor.matmul(
                        ps[:], lhsT=lhsT, rhs=S[:],
                        start=(c == 0), stop=(c == cb - 1),
                    )
                aggS = apool.tile([P, blk], bf16, tag="aggS")
                late = LATE_DVE and b >= nblk - 2
                if late:
                    nc.vector.tensor_scalar(
                        out=aggS[:], in0=ps[:], scalar1=1.0,
                        scalar2=None, op0=Alu.mult)
                else:
                    nc.scalar.activation(aggS[:], ps[:], Act.Copy)
                ph = psH.tile([P, blk], f32, tag="psH")
                nc.tensor.matmul(ph[:], lhsT=wmt[:], rhs=aggS[:],
                                 start=True, stop=True)
                if late:
                    nc.vector.tensor_scalar(
                        out=hTall[:, b * blk:(b + 1) * blk], in0=ph[:],
                        scalar1=0.0, scalar2=None, op0=Alu.max)
                else:
                    nc.scalar.activation(
                        hTall[:, b * blk:(b + 1) * blk], ph[:], Act.Relu)
                # stores: 6 groups of 7, then 42..47, then 48
                if b in (6, 13, 20, 27, 34, 41):
                    s0 = (b - 6) * blk
                    nc.sync.dma_start(houtT[:, s0:(b + 1) * blk],
                                      hTall[:, s0:(b + 1) * blk])
                elif b == 47:
                    nc.sync.dma_start(houtT[:, 42 * blk:48 * blk],
                                      hTall[:, 42 * blk:48 * blk])
                elif b == nblk - 1:
                    s0 = (nblk - 1) * blk
                    nc.sync.dma_start(houtT[:, s0:], hTall[:, s0:])
    nc.compile()
    return nc


def _assign_nodes(deg, *, ncores=NCORES, nblk=NBLK, blk=BLK):
    """Deal nodes to (core, block, pos) slots, balancing per-bin edge
    load: nodes sorted by descending degree, each placed in the
    currently least-loaded bin. Returns slot_node [ncores, nblk, blk]
    (-1 = empty) and node_slot [n] -> (core, block, pos)."""
    import heapq
    nbins = ncores * nblk
    n = len(deg)
    slot_node = np.full((nbins, blk), -1, np.int64)
    fill = np.zeros(nbins, np.int64)
    order = np.argsort(-deg, kind="stable")
    # the last block of each core gets the lowest-degree nodes, so the
    # final block on the critical-path tail has very few edge chunks
    light = order[-ncores * blk:]
    main = order[:-ncores * blk]
    heap = [(0, b) for b in range(nbins) if (b + 1) % nblk != 0]
    heapq.heapify(heap)
    for node in main:
        load, b = heapq.heappop(heap)
        slot_node[b, fill[b]] = node
        fill[b] += 1
        if fill[b] < blk:
            heapq.heappush(heap, (load + int(deg[node]), b))
    lheap = [(0, k * nblk + nblk - 1) for k in range(ncores)]
    heapq.heapify(lheap)
    for node in light:
        load, b = heapq.heappop(lheap)
        slot_node[b, fill[b]] = node
        fill[b] += 1
        if fill[b] < blk:
            heapq.heappush(lheap, (load + int(deg[node]), b))
    slot_node = slot_node.reshape(ncores, nblk, blk)
    node_slot = np.zeros((n, 3), np.int64)
    k, b, p = np.nonzero(slot_node >= 0)
    node_slot[slot_node[k, b, p]] = np.stack([k, b, p], axis=1)
    return slot_node, node_slot


def _preprocess(edge_index, edge_attr, *, n_nodes=N_NODES, blk=BLK,
                nblk=NBLK, ncores=NCORES):
    """Index-only host preprocessing (shared by both layers).

    Returns (chunks, slot_node, slot_src, slot_rel, drelT, wT):
      chunks:    [nblk] per-block chunk counts incl. the self chunk.
      slot_node: [ncores, nblk, blk] dst node per output slot (-1 pad).
      slot_src:  [ncores, G, P] int32 source node per msg slot
                 (n_nodes = the all-zeros pad row of the feature table).
      slot_rel:  [ncores, G, P] int16 gate row per slot into an extended
                 rel table (0..5 real, 6 zeros for pad, 7 ones for self).
      drelT/wT:  [ncores, P, G] f32 dst-in-block index and edge weight.
    """
    ei = np.asarray(edge_index)
    src = ei[0].astype(np.int64)
    dst = ei[1].astype(np.int64)
    attr = np.asarray(edge_attr).astype(np.int64)

    deg = np.bincount(dst, minlength=n_nodes)
    slot_node, node_slot = _assign_nodes(deg)

    w_e = (1.0 / np.maximum(deg, 1.0)).astype(np.float32)[dst]
    core = node_slot[dst, 0]
    b_e = node_slot[dst, 1]
    pos_e = node_slot[dst, 2].astype(np.float32)

    counts = np.zeros((ncores, nblk), np.int64)
    np.add.at(counts, (core, b_e), 1)
    chunks_real = (-(-counts // P)).max(axis=0)        # [nblk]
    chunks = chunks_real + 1                           # + self chunk
    G = int(chunks.sum())
    gstart = np.zeros(nblk, np.int64)
    gstart[1:] = np.cumsum(chunks)[:-1]

    slot_src = np.full((ncores, G, P), n_nodes, np.int32)
    slot_rel = np.full((ncores, G, P), N_REL, np.int16)   # 6 = zero gate
    drelT = np.zeros((ncores, P, G), np.float32)
    wT = np.zeros((ncores, P, G), np.float32)

    # self chunks: slot p of block b = node at (k, b, p), gate=ones, w=1
    pp = np.arange(P)
    for b in range(nblk):
        for k in range(ncores):
            nodes = slot_node[k, b]
            ok = nodes >= 0
            slot_src[k, gstart[b], ok] = nodes[ok].astype(np.int32)
            slot_rel[k, gstart[b], ok] = N_REL + 1        # 7 = ones gate
            wT[k, ok, gstart[b]] = 1.0
        drelT[:, :, gstart[b]] = pp.astype(np.float32)[None, :]

    # real edges, bucketed by (core, block), packed into chunks
    order = np.argsort(core * nblk + b_e, kind="stable")
    grp = (core * nblk + b_e)[order]
    starts = np.zeros(ncores * nblk + 1, np.int64)
    starts[1:] = np.cumsum(np.bincount(grp, minlength=ncores * nblk))
    rank = np.arange(len(order)) - starts[grp]
    k_e = core[order]
    colg = gstart[b_e[order]] + 1 + rank // P
    p_e = rank % P
    slot_src[k_e, colg, p_e] = src[order].astype(np.int32)
    slot_rel[k_e, colg, p_e] = attr[order].astype(np.int16)
    drelT[k_e, p_e, colg] = pos_e[order]
    wT[k_e, p_e, colg] = w_e[order]

    return chunks.tolist(), slot_node, slot_src, slot_rel, drelT, wT


_COMPILED = {}


def _get_nc(chunks):
    key = tuple(chunks)
    if key not in _COMPILED:
        _COMPILED[key] = _build_nc(chunks)
    return _COMPILED[key]


def kernel(x, rel_emb, W1, W2, edge_index, edge_attr, _trace=False):
    import ml_dtypes
    from concourse.bass_utils import run_bass_kernel_spmd

    bf16 = ml_dtypes.bfloat16
    x = np.ascontiguousarray(np.asarray(x, np.float32))
    rel_emb = np.ascontiguousarray(np.asarray(rel_emb, np.float32))
    W1 = np.ascontiguousarray(np.asarray(W1, np.float32))
    W2 = np.ascontiguousarray(np.asarray(W2, np.float32))

    chunks, slot_node, slot_src, slot_rel, drelT, wT = _preprocess(
        edge_index, edge_attr)
    nc = _get_nc(chunks)

    iota = np.ascontiguousarray(
        np.broadcast_to(np.arange(BLK, dtype=np.float32), (P, BLK))
    ).astype(bf16)
    # extended gate table: 6 real rels, zeros (pad), ones (self)
    relg = np.vstack([rel_emb, np.zeros((1, D), np.float32),
                      np.ones((1, D), np.float32)]).astype(bf16)
    # scatter map: output row slot_node[k,b,p] <- houtT column b*BLK+p
    flat_nodes = slot_node.reshape(NCORES, -1)

    results = []

    w_slot = wT.transpose(0, 2, 1)  # [ncores, G, P]

    def run_layer(tab_bf, W):
        in_maps = []
        for k in range(NCORES):
            # msg slot (g, p) = tab[src] * relg[rel] * w  -> dmsg[p, g, :]
            m = (tab_bf[slot_src[k]].astype(np.float32)
                 * relg[slot_rel[k]].astype(np.float32)
                 * w_slot[k][:, :, None]).astype(bf16)
            in_maps.append(dict(
                dmsg=np.ascontiguousarray(m.transpose(1, 0, 2)),
                drel=drelT[k], dwm=W.astype(bf16)))
        res = run_bass_kernel_spmd(
            nc, in_maps, core_ids=list(range(NCORES)), trace=False)
        results.append(res)
        h = np.zeros((N_NODES, D), np.float32)
        for k in range(NCORES):
            cols = np.asarray(res.results[k]["houtT"]).T  # [SHARD, D] bf16
            sel = flat_nodes[k] >= 0
            h[flat_nodes[k][sel]] = cols[sel].astype(np.float32)
        return h

    def pad_tab(t_f32):
        return np.vstack([t_f32, np.zeros((1, D), np.float32)]).astype(bf16)

    h1 = run_layer(pad_tab(x), W1)
    h2 = run_layer(pad_tab(h1), W2)
    out = np.ascontiguousarray(h2, dtype=np.float32)
    if _trace:
        kernel._last_results = results
    return out



# revision 4
# speedup vs baseline: 1.5289x; 1.5289x over previous
"""Trainium2 Bass kernel for a 2-layer relational GNN (ConceptGNN).

Math per layer (reference):
    msg  = x[src] * rel_emb[edge_attr]               # [E, D]
    agg  = segment_sum(msg, dst) / max(deg, 1)       # [N, D] mean
    h    = relu((x + agg) @ W)

Distribution: destination-sharded across 8 NeuronCores, 49 blocks of
128 dst slots per core. Nodes are grouped into blocks by DEGREE
(sorted descending), so every dst slot in a block has (nearly) the
same number of edges. Edge slot r of dst p lands at chunk r, column p
— which makes the one-hot aggregation matrix of EVERY chunk the
IDENTITY. No per-chunk one-hot builds are needed: the host bakes
1/deg into the message values and aggregation is
    aggT_b = sum_c msgT_c  (via PE matmuls against a constant identity)

Messages stream in fp8 (e4m3) and are accumulated two chunks at a
time with DoubleRow fp8 matmuls (0.5 cycles/row). Low-degree blocks
(chunk count <= KDEG) stream in bf16 instead: their aggregates average
few terms, so fp8 noise there dominates the absmax error. Each block
also has a bf16 self chunk carrying x rows (the residual, weight 1).
Dense phase per block: hT = relu(W.T @ (x+agg)T) in bf16 with f32 PSUM.

The same compiled program runs twice (layer 1 from x, layer 2 from h1);
edges are identical across layers so the slot structure is shared.
"""

import numpy as np

# ---- problem constants (fixed by the harness contract) ----
N_NODES = 50000
N_EDGES = 640000
D = 128
N_REL = 6
NCORES = 8
P = 128

BLK = 128                 # dst-node block width
NBLK = 49                 # blocks (positions) per core; 8*49*128 = 50176
SHARD = NBLK * BLK        # 6272 dst slots per core
KDEG = 8                  # blocks with chunks <= KDEG stream bf16
LA = 7                    # DMA lookahead (positions)


def _build_nc(chunks, npair, *, d=D, nblk=NBLK, blk=BLK, ncores=NCORES):
    """Build the single-layer Bass program.

    chunks[j]: edge-chunk count of position j (excl. self chunk).
    npair[j]:  fp8 DoubleRow pair count (0 => bf16 position).
    Identical across cores so one SPMD program serves all 8.
    """
    import concourse.bass as bass
    import concourse.bacc as bacc
    import concourse.mybir as mybir
    import concourse.tile as tile

    f32 = mybir.dt.float32
    bf16 = mybir.dt.bfloat16
    f8 = mybir.dt.float8e4
    Act = mybir.ActivationFunctionType
    Alu = mybir.AluOpType
    PM = mybir.MatmulPerfMode

    chunks = [int(c) for c in chunks]
    npair = [int(c) for c in npair]
    NP = sum(npair)
    # dmb row count: one self chunk per position + bf16 edge chunks
    nb = [1 + (chunks[j] if npair[j] == 0 else 0) for j in range(nblk)]
    GB = sum(nb)

    nc = bacc.Bacc("TRN2", target_bir_lowering=False, debug=False,
                   num_devices=ncores)
    dm8 = nc.dram_tensor("dm8", [P, max(NP, 1), 2, d], f8,
                         kind="ExternalInput")
    dmb = nc.dram_tensor("dmb", [P, GB, d], bf16, kind="ExternalInput")
    dwm = nc.dram_tensor("dwm", [d, d], bf16, kind="ExternalInput")
    did2 = nc.dram_tensor("did2", [P, 2, blk], f8, kind="ExternalInput")
    didb = nc.dram_tensor("didb", [P, blk], bf16, kind="ExternalInput")
    houtT = nc.dram_tensor("houtT", [P, nblk * blk], bf16,
                           kind="ExternalOutput")

    p8 = np.concatenate([[0], np.cumsum(npair)]).tolist()
    pb = np.concatenate([[0], np.cumsum(nb)]).tolist()

    G4 = 4                 # positions per PSUM group (shared copy/relu)
    ngrp = (nblk + G4 - 1) // G4

    with tile.TileContext(nc) as tc:
        with (
            tc.tile_pool(name="const", bufs=1) as cpool,
            tc.tile_pool(name="m8", bufs=2 * LA) as m8pool,
            tc.tile_pool(name="mb", bufs=2 * LA) as mbpool,
            tc.tile_pool(name="agg", bufs=3) as apool,
            tc.tile_pool(name="psA", bufs=3, space="PSUM") as psA,
            tc.tile_pool(name="psH", bufs=2, space="PSUM") as psH,
        ):
            id2 = cpool.tile([P, 2, blk], f8, tag="id2")
            idb = cpool.tile([P, blk], bf16, tag="idb")
            wmt = cpool.tile([P, d], bf16, tag="wm")
            hTall = cpool.tile([P, nblk * blk], bf16, tag="hT")
            nc.scalar.dma_start(id2[:], did2[:])
            nc.scalar.dma_start(idb[:], didb[:])
            nc.scalar.dma_start(wmt[:], dwm[:])

            m8t, mbt = {}, {}
            # msg DMAs rotate across the three DMA-capable queues
            qs = [nc.sync, nc.scalar, nc.gpsimd]

            def load(j):
                q = qs[j % 3]
                qb = qs[(j + 1) % 3]
                if npair[j] > 0:
                    mg = m8pool.tile([P, max(npair), 2, d], f8, tag="mg8")
                    q.dma_start(mg[:, :npair[j]], dm8[:, p8[j]:p8[j + 1]])
                    m8t[j] = mg
                mg = mbpool.tile([P, max(nb), d], bf16, tag="mgb")
                qb.dma_start(mg[:, :nb[j]], dmb[:, pb[j]:pb[j + 1]])
                mbt[j] = mg

            for j in range(min(LA, nblk)):
                load(j)

            ps = None
            for j in range(nblk):
                g0 = (j // G4) * G4              # first position of group
                gn = min(G4, nblk - g0)          # group size
                gi = j - g0
                if gi == 0:
                    ps = psA.tile([P, G4 * blk], f32, tag="psA")
                if j + LA < nblk:
                    load(j + LA)
                mgb = mbt.pop(j)
                pj = ps[:, gi * blk:(gi + 1) * blk]
                last = (chunks[j] == 0)
                nc.tensor.matmul(pj, lhsT=mgb[:, 0], rhs=idb[:],
                                 start=True, stop=last)
                if npair[j] > 0:
                    mg8 = m8t.pop(j)
                    for c in range(npair[j]):
                        nc.tensor.matmul(
                            pj, lhsT=mg8[:, c], rhs=id2[:],
                            perf_mode=PM.DoubleRow,
                            start=False, stop=(c == npair[j] - 1))
                else:
                    for c in range(chunks[j]):
                        nc.tensor.matmul(
                            pj, lhsT=mgb[:, 1 + c], rhs=idb[:],
                            start=False, stop=(c == chunks[j] - 1))
                if gi != gn - 1:
                    continue
                # end of group: wide copy, dense matmuls, wide relu
                wide = gn * blk
                aggS = apool.tile([P, G4 * blk], bf16, tag="aggS")
                nc.vector.tensor_scalar(out=aggS[:, :wide],
                                        in0=ps[:, :wide],
                                        scalar1=1.0, scalar2=None,
                                        op0=Alu.mult)
                ph = psH.tile([P, G4 * blk], f32, tag="psH")
                for i in range(gn):
                    nc.tensor.matmul(ph[:, i * blk:(i + 1) * blk],
                                     lhsT=wmt[:],
                                     rhs=aggS[:, i * blk:(i + 1) * blk],
                                     start=True, stop=True)
                hcol = hTall[:, g0 * blk:(g0 + gn) * blk]
                if (j // G4) % 2 == 0:
                    nc.vector.tensor_scalar(out=hcol, in0=ph[:, :wide],
                                            scalar1=0.0, scalar2=None,
                                            op0=Alu.max)
                else:
                    nc.scalar.activation(hcol, ph[:, :wide], Act.Relu)
                # store finished groups (three batches)
                if j // G4 in (3, 7, 11):
                    g1 = (j // G4 - 3) * G4 * blk
                    nc.sync.dma_start(
                        houtT[:, g1:(g0 + gn) * blk],
                        hTall[:, g1:(g0 + gn) * blk])
                elif j == nblk - 1:
                    nc.sync.dma_start(houtT[:, 12 * G4 * blk:],
                                      hTall[:, 12 * G4 * blk:])
    nc.compile()
    return nc


def _preprocess(edge_index, edge_attr, *, n_nodes=N_NODES, blk=BLK,
                nblk=NBLK, ncores=NCORES):
    """Index-only host preprocessing (shared by both layers).

    Degree-sorted block assignment: node order[i] -> core i%8 wait no:
    block b = order[128b:128b+128]; block b -> (core b%8, position b//8).
    Every node's edges occupy slots (chunk r, column p) of its block.
    """
    ei = np.asarray(edge_index)
    src = ei[0].astype(np.int64)
    dst = ei[1].astype(np.int64)
    attr = np.asarray(edge_attr).astype(np.int64)

    deg = np.bincount(dst, minlength=n_nodes)
    order = np.argsort(-deg, kind="stable")
    nslot = ncores * nblk * blk
    slot_node = np.full(nslot, -1, np.int64)
    slot_node[:n_nodes] = order
    # slot i -> block i//128; block b -> (position b//8, core b%8, col i%128)
    slot_node = slot_node.reshape(nblk, ncores, blk)   # [j, k, p]

    # per-position chunk count = max degree among its 8 blocks
    chunks = np.zeros(nblk, np.int64)
    for j in range(nblk):
        nodes = slot_node[j]
        valid = nodes >= 0
        if valid.any():
            chunks[j] = deg[nodes[valid]].max()
    is8 = chunks > KDEG
    npair = np.where(is8, (chunks + 1) // 2, 0)
    nb = 1 + np.where(is8, 0, chunks)
    NP = int(npair.sum())
    GB = int(nb.sum())
    p8 = np.concatenate([[0], np.cumsum(npair)])
    pb = np.concatenate([[0], np.cumsum(nb)])

    # node -> (position j, core k, column p)
    node_j = np.zeros(n_nodes, np.int64)
    node_k = np.zeros(n_nodes, np.int64)
    node_p = np.zeros(n_nodes, np.int64)
    jj, kk, pp = np.nonzero(slot_node >= 0)
    node_j[slot_node[jj, kk, pp]] = jj
    node_k[slot_node[jj, kk, pp]] = kk
    node_p[slot_node[jj, kk, pp]] = pp

    # per-edge rank within its dst
    order_e = np.argsort(dst, kind="stable")
    cnt = np.bincount(dst, minlength=n_nodes)
    starts = np.zeros(n_nodes + 1, np.int64)
    starts[1:] = np.cumsum(cnt)
    rank = np.empty(len(dst), np.int64)
    rank[order_e] = np.arange(len(dst)) - starts[dst[order_e]]

    j_e = node_j[dst]
    k_e = node_k[dst]
    p_e = node_p[dst]
    e8 = is8[j_e]

    # flat row index into dm8[k].reshape(P*NP*2, D):
    #   ((p * NP + p8[j] + rank//2) * 2 + rank%2)
    r8 = ((p_e * NP + p8[j_e] + rank // 2) * 2 + rank % 2)[e8]
    k8 = k_e[e8]
    # flat row index into dmb[k].reshape(P*GB, D): p * GB + pb[j] + 1 + rank
    rb = (p_e * GB + pb[j_e] + 1 + rank)[~e8]
    kb = k_e[~e8]
    # self rows: for each (j, k, p): row p*GB + pb[j]
    selfrow = (np.arange(blk)[None, :] * GB + pb[:nblk, None])  # [j, p]

    w_e = (1.0 / np.maximum(deg, 1.0)).astype(np.float32)[dst]

    return dict(
        chunks=chunks.tolist(), npair=npair.tolist(), NP=NP, GB=GB,
        slot_node=slot_node, selfrow=selfrow,
        src=src, attr=attr, w_e=w_e, e8=e8,
        r8=r8, k8=k8, rb=rb, kb=kb,
    )


_COMPILED = {}


def _get_nc(chunks, npair):
    key = (tuple(chunks), tuple(npair))
    if key not in _COMPILED:
        _COMPILED[key] = _build_nc(chunks, npair)
    return _COMPILED[key]


def kernel(x, rel_emb, W1, W2, edge_index, edge_attr, _trace=False):
    import ml_dtypes
    from concourse.bass_utils import run_bass_kernel_spmd

    bf16 = ml_dtypes.bfloat16
    f8 = ml_dtypes.float8_e4m3fn
    x = np.ascontiguousarray(np.asarray(x, np.float32))
    rel_emb = np.ascontiguousarray(np.asarray(rel_emb, np.float32))
    W1 = np.ascontiguousarray(np.asarray(W1, np.float32))
    W2 = np.ascontiguousarray(np.asarray(W2, np.float32))

    pre = _preprocess(edge_index, edge_attr)
    NP, GB = pre["NP"], pre["GB"]
    nc = _get_nc(pre["chunks"], pre["npair"])

    eye8 = np.zeros((P, 2, BLK), f8)
    eye8[np.arange(P), 0, np.arange(BLK)] = 1
    eye8[np.arange(P), 1, np.arange(BLK)] = 1
    eyeb = np.eye(P, BLK).astype(bf16)

    src, attr, w_e, e8 = pre["src"], pre["attr"], pre["w_e"], pre["e8"]
    r8, k8, rb, kb = pre["r8"], pre["k8"], pre["rb"], pre["kb"]
    slot_node, selfrow = pre["slot_node"], pre["selfrow"]

    results = []

    def run_layer(tab, W):
        # tab: [N_NODES, D] float32 node features for this layer
        msg = tab[src] * rel_emb[attr] * w_e[:, None]
        msg8 = msg[e8].astype(f8)
        msgb = msg[~e8].astype(bf16)
        Wb = W.astype(bf16)
        in_maps = []
        for k in range(NCORES):
            m8 = np.zeros((P * NP * 2, D), f8)
            m8[r8[k8 == k]] = msg8[k8 == k]
            mb = np.zeros((P * GB, D), bf16)
            mb[rb[kb == k]] = msgb[kb == k]
            nodes = slot_node[:, k, :]          # [j, p]
            valid = nodes >= 0
            selfvals = np.zeros((NBLK, BLK, D), np.float32)
            selfvals[valid] = tab[nodes[valid]]
            mb[selfrow.ravel()] = selfvals.reshape(-1, D).astype(bf16)
            in_maps.append(dict(
                dm8=m8.reshape(P, NP, 2, D),
                dmb=mb.reshape(P, GB, D),
                dwm=Wb, did2=eye8, didb=eyeb))
        res = run_bass_kernel_spmd(
            nc, in_maps, core_ids=list(range(NCORES)), trace=False)
        results.append(res)
        h = np.zeros((N_NODES, D), np.float32)
        for k in range(NCORES):
            cols = np.asarray(res.results[k]["houtT"]).T  # [SHARD, D] bf16
            nodes = slot_node[:, k, :].ravel()
            sel = nodes >= 0
            h[nodes[sel]] = cols[sel].astype(np.float32)
        return h

    h1 = run_layer(x, W1)
    h2 = run_layer(h1, W2)
    out = np.ascontiguousarray(h2, dtype=np.float32)
    if _trace:
        kernel._last_results = results
    return out
